# revision 20
# baseline (speedup 1.0000x reference)
"""Trainium2 Bass kernel for nn_CMAF (cross-modal attention fusion block).

Feature-major layout: every activation tile is [128 features x 1024
samples]; all matmuls are weight-stationary bf16 with batch as the
moving free dim.  Inputs are cast bf16 host-side and DMA-transposed in.

The elementwise work (not the PE) is the bottleneck for this model, so
ops are spread across all four compute engines per 1024-sample block:
  - ACT: Square(z+b), per-LN exp(-0.5*ln(var)), attention Tanh, Gelu,
    gate Exp + exp(-ln(Z)).  Everything lands in two ACT tables per
    block (natural_log_exp, then gelu+tanh) = 2 table loads/block.
    The s3/s5 LN groups share one wide Exp over a 3-slice tile.
  - DVE: PSUM-coupled fused ops (scalar_tensor_tensor), q PSUM->SBUF
    copy, gate normalize multiplies.
  - GPSIMD/Pool (otherwise idle): SBUF-only squares, LN applies, gate
    diffs.
  - PE: projections, q/k, dv, head-sum score matmul, out-proj, FFN, LN
    partition reductions (ones-matmul, on a dedicated 1-deep PSUM ring
    separate from the 3-deep data ring), gate broadcasts.

Algebraic folds (host-side, float64):
  - LayerNorm mean-subtraction folded into producing weights
    (C = I - 11^T/128); eps dropped (var ~ O(1) >> 1e-5).
  - 2-way attention softmax -> a0 = (1+tanh(q.dk/(2 sqrt(dh))))/2; the
    0.5 folded into Wo; the v1 path folded as Wov = C.Wo.Wv applied
    directly to P[s1], so v1 is never materialized.
  - LN rsqrt = exp(-0.5*ln(var)) on ACT: AF.Rsqrt is blocked in bass,
    custom-DVE ops fail this walrus, and hw reciprocal is ~6 cyc/elem.
  - gate softmax: fused = x2[2] + en0*(x2[0]-x2[2]) + en1*(x2[1]-x2[2]).

Output is written feature-major bf16 [128 x Bc]; the host transposes
back to [B, 128] f32 during the gather/unshard step.

Data parallel over 8 NeuronCores: 8192 samples each.
"""

import numpy as np
import ml_dtypes

import concourse.bass as bass
import concourse.mybir as mybir
from concourse.tile import TileContext
from concourse.vector_clock import ScopedClock
from concourse.bass_utils import run_bass_kernel_spmd

F32 = mybir.dt.float32
BF16 = mybir.dt.bfloat16
AL = mybir.AluOpType
AF = mybir.ActivationFunctionType
NPBF = ml_dtypes.bfloat16

D = 128
SP = 1280
FFN = 256
NB = 3
DH = 32
KV_IDX = ((1, 2), (0, 2), (0, 1))
NCORES = 8
BLK = 1024
MMN = 512
ISQ = float(1.0 / np.sqrt(DH))


def _patch_tile_drain():
    """walrus rejects >4 sem waits on one instruction; Tile's tail drain
    carries one wait per logical proc.  Re-emit them as standalone
    wait_ge instructions ahead of the drain."""
    TC = TileContext
    if getattr(TC, "_drain_patched", False):
        return

    def patched(self, tick_clock, wait_clock):
        nop_inst = self.nc.sync.nop()
        wait_clock.add_sem_waits(
            nop_inst.ins, ScopedClock({None: tick_clock.global_clock})
        )
        d = nop_inst.ins
        si = d.sync_info
        waits = list(si.on_wait) if si is not None else []
        if len(waits) > 4:
            si.on_wait = []
            d.sync_info = si
            name2sem = {s.name: s for s in self.sems.allocated().values()}
            for w in waits:
                sem = name2sem.get(w.ant_name)
                if sem is None:
                    raise RuntimeError(f"drain patch: unknown sem {w.ant_name}")
                self.nc.sync.wait_ge(sem, w.wait_value)
        self.nc.sync.drain()
        self.nc.all_engine_barrier()
        popped = self.nc._tile_sem_poison_stack.pop()
        assert popped is self._sem_poison
        self.nc.clear_and_free_semaphores(list(self.sems.allocated().values()))
        self.nc.all_engine_barrier()

    TC._drain_and_barrier = patched
    TC._drain_patched = True


def _fix_wait_overflow(nc):
    """walrus enforces per-opcode caps on sync-wait commands attached to
    one instruction.  Move the excess onto same-engine NOPs inserted
    immediately before the instruction."""
    LIMITS = {}
    DEFAULT_LIM = 1
    for fn in nc.m.functions:
        for bb in fn.blocks:
            insts = list(bb.instructions)
            out = []
            changed = False
            for inst in insts:
                si = getattr(inst, "sync_info", None)
                w = list(si.on_wait) if si is not None and si.on_wait else []
                lim = LIMITS.get(type(inst).__name__, DEFAULT_LIM)
                if len(w) > lim:
                    excess = w[lim:]
                    keep = w[:lim]
                    eng = nc.engines[inst.engine]
                    nops = []
                    for i in range(0, len(excess), 1):
                        chunk = excess[i:i + 1]
                        nop_bi = eng.nop()
                        nop_inst = nop_bi.ins
                        cb = nc.cur_bb.bb
                        cb.instructions = [x for x in cb.instructions
                                           if x.name != nop_inst.name]
                        import bass_rust
                        nop_inst.sync_info = bass_rust.SyncInfo(
                            on_wait=chunk, on_update=[])
                        nops.append(nop_inst)
                    si.on_wait = keep
                    inst.sync_info = si
                    out.extend(nops)
                    changed = True
                out.append(inst)
            if changed:
                bb.instructions = out


def prep_weights(inp):
    """Host-side prep of all weights into SBUF layouts. bf16 for matmul
    operands, fp32 for per-partition bias vectors."""
    f64 = np.float64
    C = np.eye(D, dtype=f64) - 1.0 / D

    def bf(a):
        return np.ascontiguousarray(a.astype(np.float32)).astype(NPBF)

    def f32(a):
        return np.ascontiguousarray(a, dtype=np.float32)

    w = {}
    wsp = C @ inp["proj_w_spatial"].astype(f64)            # [128,1280]
    w["wspT"] = bf(np.transpose(wsp.reshape(D, 10, D), (2, 1, 0)).reshape(D, 10 * D))
    wgf = np.stack([C @ inp["proj_w_gf"][i].astype(f64) for i in range(2)])
    w["wgfT"] = bf(np.transpose(wgf, (2, 0, 1)).reshape(D, 2 * D))
    w["bc"] = f32(C @ inp["proj_b"].astype(f64).T)         # [128,3]
    w["emb"] = f32(inp["mod_emb"].T)

    ipw = inp["in_proj_w"].astype(f64)                     # [3, 384, 128]
    wq, wk, wv = ipw[:, :D], ipw[:, D:2 * D], ipw[:, 2 * D:]
    w["wqT"] = bf(np.transpose(wq, (2, 0, 1)).reshape(D, NB * D))
    w["wkT"] = bf(np.transpose(wk, (2, 0, 1)).reshape(D, NB * D))
    w["wvT"] = bf(np.transpose(wv, (2, 0, 1)).reshape(D, NB * D))
    wo2 = np.stack([0.5 * (C @ inp["out_proj_w"][n].astype(f64))
                    for n in range(NB)])
    w["wo2T"] = bf(np.transpose(wo2, (2, 0, 1)).reshape(D, NB * D))
    wov = np.stack([C @ inp["out_proj_w"][n].astype(f64) @ wv[n]
                    for n in range(NB)])
    w["wovT"] = bf(np.transpose(wov, (2, 0, 1)).reshape(D, NB * D))
    ob2 = np.stack([
        C @ inp["out_proj_b"][n].astype(f64)
        - inp["mod_emb"][n].astype(f64).mean()
        for n in range(NB)])
    w["ob2"] = f32(ob2.T)

    w1 = inp["ffn_w1"].astype(f64)                         # [3, 256, 128]
    w["w1T"] = bf(np.transpose(w1, (2, 0, 1)).reshape(D, NB * FFN))
    w["b1"] = f32(inp["ffn_b1"].reshape(NB * 2, D).T)      # [128, 6]
    w2 = np.stack([C @ inp["ffn_w2"][n].astype(f64) for n in range(NB)])
    w2c = w2.reshape(NB, D, 2, D)                          # [n, j, c, p]
    w["w2T"] = bf(np.transpose(w2c, (3, 0, 2, 1)).reshape(D, NB * 2 * D))
    b2c = np.stack([C @ inp["ffn_b2"][n].astype(f64) for n in range(NB)])
    w["b2c"] = f32(b2c.T)

    gw = inp["gate_w"].astype(f64).reshape(NB, NB, D)      # [j, n, p]
    w["gwT"] = bf(np.transpose(gw, (2, 1, 0)).reshape(D, NB * NB))
    w["gateb"] = f32(inp["gate_b"].reshape(NB, 1))

    w["onesT"] = bf(np.full((D, D), 1.0 / D))
    hs = np.zeros((D, D), dtype=np.float32)
    for h in range(4):
        hs[h * DH:(h + 1) * DH, h * DH:(h + 1) * DH] = 1.0
    w["hsel"] = bf(hs)
    w["ones3"] = bf(np.ones((NB, NB)))
    esel2 = np.zeros((NB, 2 * D), dtype=np.float32)
    for n in range(2):
        esel2[n, n * D:(n + 1) * D] = 1.0
    w["esel2"] = bf(esel2)

    assert np.allclose(inp["proj_ln_g"], 1) and np.allclose(inp["proj_ln_b"], 0)
    assert np.allclose(inp["attn_ln_g"], 1) and np.allclose(inp["attn_ln_b"], 0)
    assert np.allclose(inp["ffn_ln_g"], 1) and np.allclose(inp["ffn_ln_b"], 0)
    assert np.allclose(inp["in_proj_b"], 0)
    return w


WEIGHT_SPECS = {
    "wspT": ((D, 10 * D), BF16), "wgfT": ((D, 2 * D), BF16),
    "bc": ((D, NB), F32), "emb": ((D, NB), F32),
    "wqT": ((D, NB * D), BF16), "wkT": ((D, NB * D), BF16),
    "wvT": ((D, NB * D), BF16), "wo2T": ((D, NB * D), BF16),
    "wovT": ((D, NB * D), BF16), "ob2": ((D, NB), F32),
    "w1T": ((D, NB * FFN), BF16), "b1": ((D, NB * 2), F32),
    "w2T": ((D, NB * 2 * D), BF16), "b2c": ((D, NB), F32),
    "gwT": ((D, NB * NB), BF16), "gateb": ((NB, 1), F32),
    "onesT": ((D, D), BF16), "hsel": ((D, D), BF16),
    "ones3": ((NB, NB), BF16), "esel2": ((NB, 2 * D), BF16),
}


def build_program(Bc, repeat=1):
    nc = bass.Bass()
    xsp = nc.dram_tensor("x_spatial", [Bc, SP], BF16, kind="ExternalInput")
    xg = nc.dram_tensor("x_gradient", [Bc, D], BF16, kind="ExternalInput")
    xf = nc.dram_tensor("x_frequency", [Bc, D], BF16, kind="ExternalInput")
    wd = {k: nc.dram_tensor(k, list(s[0]), s[1], kind="ExternalInput")
          for k, s in WEIGHT_SPECS.items()}
    out = nc.dram_tensor("out", [D, Bc], BF16, kind="ExternalOutput")

    nblk = Bc // BLK
    assert Bc % BLK == 0

    with TileContext(nc) as tc, nc.allow_low_precision(reason="bf16 kernel"):
        with (
            tc.tile_pool(name="wp", bufs=1) as wp,
            tc.tile_pool(name="xin", bufs=2) as xin,
            tc.tile_pool(name="work", bufs=2) as wk_,
            tc.tile_pool(name="ps", bufs=3, space="PSUM") as psp,
            tc.tile_pool(name="psm", bufs=1, space="PSUM") as psm,
        ):
            W = {}
            for k, s in WEIGHT_SPECS.items():
                W[k] = wp.tile(list(s[0]), s[1], tag=k, name=k)
                nc.gpsimd.dma_start(W[k][:], wd[k][:])

            def mm(out_ap, lhsT, rhs, start=True, stop=True):
                for h in range(BLK // MMN):
                    nc.tensor.matmul(out_ap[:, h * MMN:(h + 1) * MMN], lhsT,
                                     rhs[:, h * MMN:(h + 1) * MMN],
                                     start=start, stop=stop)

            def ln_group():
                """Per-stage LN scale pipeline: three Ln ops write slices of
                one [D, NB, BLK] tile; a single wide Exp computes all three
                1/sqrt(v) = exp(-0.5*ln(v)) broadcasts at once (saves two
                ACT issue overheads per stage; ln/exp stay in the
                natural_log_exp table -- hw reciprocal is ~6 cyc/elem)."""
                lnv = wk_.tile([D, NB, BLK], BF16, tag="rv", bufs=1)
                rb = wk_.tile([D, NB, BLK], BF16, tag="rb", bufs=1)

                def emit_ln(n, mq_ps):
                    nc.scalar.activation(lnv[:, n, :], mq_ps[:], AF.Ln)

                def emit_exp():
                    nc.scalar.activation(rb[:], lnv[:], AF.Exp, scale=-0.5)
                return rb, emit_ln, emit_exp

            def s0_load(b):
                r0 = (b % nblk) * BLK
                st = {}
                xspT = xin.tile([D, 10 * BLK], BF16, tag="xspT")
                nc.sync.dma_start(
                    xspT[:].rearrange("p (c n) -> p c n", c=10),
                    xsp[r0:r0 + BLK, :], transpose=True)
                st["xspT"] = xspT
                st["xgT"] = xin.tile([D, BLK], BF16, tag="xgT", name="xgT")
                nc.sync.dma_start(st["xgT"][:], xg[r0:r0 + BLK, :], transpose=True)
                st["xfT"] = xin.tile([D, BLK], BF16, tag="xfT", name="xfT")
                nc.sync.dma_start(st["xfT"][:], xf[r0:r0 + BLK, :], transpose=True)
                return st

            def s1_proj(st):
                P = [None] * NB
                for n in (0, 1, 2):
                    z = psp.tile([D, BLK], F32, tag="ps")
                    if n == 0:
                        for c in range(10):
                            mm(z[:], W["wspT"][:, c * D:(c + 1) * D],
                               st["xspT"][:, c * BLK:(c + 1) * BLK],
                               start=(c == 0), stop=(c == 9))
                    else:
                        key = "xgT" if n == 1 else "xfT"
                        mm(z[:], W["wgfT"][:, (n - 1) * D:n * D], st[key][:])
                    sqa = wk_.tile([D, BLK], BF16, tag="sqx", bufs=2)
                    nc.scalar.activation(sqa[:], z[:], AF.Square,
                                         bias=W["bc"][:, n:n + 1])
                    mq = psm.tile([D, BLK], F32, tag="mq")
                    mm(mq[:], W["onesT"][:], sqa[:])
                    lnv = wk_.tile([D, BLK], BF16, tag="rv1", bufs=1)
                    nc.scalar.activation(lnv[:], mq[:], AF.Ln)
                    rb = wk_.tile([D, BLK], BF16, tag="rb1", bufs=1)
                    nc.scalar.activation(rb[:], lnv[:], AF.Exp, scale=-0.5)
                    p_ = wk_.tile([D, BLK], BF16, tag=f"P{n}", bufs=2)
                    nc.vector.scalar_tensor_tensor(
                        p_[:], z[:], W["bc"][:, n:n + 1], rb[:],
                        AL.add, AL.mult)
                    nc.vector.tensor_scalar_add(p_[:], p_[:], W["emb"][:, n:n + 1])
                    P[n] = p_
                st["P"] = P
                dP = []
                for n in range(NB):
                    s0_, s1_ = KV_IDX[n]
                    dp = wk_.tile([D, BLK], BF16, tag=f"dP{n}", bufs=2)
                    nc.gpsimd.tensor_tensor(dp[:], P[s0_][:], P[s1_][:],
                                            AL.subtract)
                    dP.append(dp)
                st["dP"] = dP

            def s3_ln2(st):
                u = st["u"]
                rb2, emit_ln, emit_exp = ln_group()
                for n in range(NB):
                    sq2 = wk_.tile([D, BLK], BF16, tag="sqx", bufs=2)
                    nc.gpsimd.tensor_tensor(sq2[:], u[n][:], u[n][:], AL.mult)
                    mq2 = psm.tile([D, BLK], F32, tag="mq")
                    mm(mq2[:], W["onesT"][:], sq2[:])
                    emit_ln(n, mq2)
                emit_exp()
                x1 = []
                for n in range(NB):
                    x1n = wk_.tile([D, BLK], BF16, tag=f"x1{n}", bufs=2)
                    nc.gpsimd.tensor_tensor(x1n[:], u[n][:], rb2[:, n, :],
                                            AL.mult)
                    x1.append(x1n)
                st["x1"] = x1

            def s5_ln3(st):
                x2p, sq3 = st["x2p"], st["sq3"]
                x2all = wk_.tile([D, NB, BLK], BF16, tag="x2all", bufs=2)
                rb3, emit_ln, emit_exp = ln_group()
                for n in range(NB):
                    mq3 = psm.tile([D, BLK], F32, tag="mq")
                    mm(mq3[:], W["onesT"][:], sq3[n][:])
                    emit_ln(n, mq3)
                emit_exp()
                for n in range(NB):
                    nc.gpsimd.tensor_tensor(x2all[:, n, :], x2p[n][:],
                                            rb3[:, n, :], AL.mult)
                st["x2all"] = x2all

            def s2a_qk(st):
                P, dP = st["P"], st["dP"]
                t0s = []
                for n in range(NB):
                    q_ps = psp.tile([D, BLK], F32, tag="ps")
                    mm(q_ps[:], W["wqT"][:, n * D:(n + 1) * D], P[n][:])
                    dk_ps = psp.tile([D, BLK], F32, tag="ps")
                    mm(dk_ps[:], W["wkT"][:, n * D:(n + 1) * D], dP[n][:])
                    qsb = wk_.tile([D, BLK], BF16, tag="qsb", bufs=2)
                    nc.vector.tensor_copy(qsb[:], q_ps[:])
                    t0 = wk_.tile([D, BLK], BF16, tag="t0", bufs=3)
                    nc.vector.tensor_tensor(t0[:], qsb[:], dk_ps[:], AL.mult)
                    t0s.append(t0)
                st["t0"] = t0s

            def s4_ffn_mm(st):
                x1 = st["x1"]
                fps = []
                for n in range(NB):
                    h_sb = wk_.tile([D, 2, BLK], BF16, tag="hh", bufs=1)
                    for c in range(2):
                        h_ps = psp.tile([D, BLK], F32, tag="ps")
                        mm(h_ps[:],
                           W["w1T"][:, n * FFN + c * D: n * FFN + (c + 1) * D],
                           x1[n][:])
                        nc.scalar.activation(
                            h_sb[:, c, :], h_ps[:], AF.Gelu,
                            bias=W["b1"][:, 2 * n + c: 2 * n + c + 1])
                    f_ps = psp.tile([D, BLK], F32, tag="ps")
                    for c in range(2):
                        mm(f_ps[:], W["w2T"][:, (2 * n + c) * D:(2 * n + c + 1) * D],
                           h_sb[:, c, :], start=(c == 0), stop=(c == 1))
                    fps.append(f_ps)
                st["fps"] = fps

            def s4_ffn_post(st):
                x1 = st["x1"]
                x2p = []
                sq3 = []
                for n in range(NB):
                    x2pn = wk_.tile([D, BLK], BF16, tag=f"x2p{n}", bufs=2)
                    nc.vector.scalar_tensor_tensor(
                        x2pn[:], st["fps"][n][:], W["b2c"][:, n:n + 1],
                        x1[n][:], AL.add, AL.add)
                    x2p.append(x2pn)
                    sq3n = wk_.tile([D, BLK], BF16, tag=f"sq3{n}", bufs=2)
                    nc.gpsimd.tensor_tensor(sq3n[:], x2pn[:], x2pn[:], AL.mult)
                    sq3.append(sq3n)
                st["x2p"] = x2p
                st["sq3"] = sq3
                del st["fps"]

            def s2b_score(st):
                t0s = st["t0"]
                ths = []
                for n in range(NB):
                    d_ps = psp.tile([D, BLK], F32, tag="ps")
                    mm(d_ps[:], W["hsel"][:], t0s[n][:])
                    th = wk_.tile([D, BLK], BF16, tag="th", bufs=3)
                    nc.scalar.activation(th[:], d_ps[:], AF.Tanh, scale=ISQ / 2)
                    ths.append(th)
                st["th"] = ths

            def s2b_out(st):
                P, dP, ths = st["P"], st["dP"], st["th"]
                us = []
                for n in range(NB):
                    s0_, s1_ = KV_IDX[n]
                    dv_ps = psp.tile([D, BLK], F32, tag="ps")
                    mm(dv_ps[:], W["wvT"][:, n * D:(n + 1) * D], dP[n][:])
                    tp = wk_.tile([D, BLK], BF16, tag="tp", bufs=2)
                    nc.vector.scalar_tensor_tensor(
                        tp[:], ths[n][:], 1.0, dv_ps[:], AL.add, AL.mult)
                    o_ps = psp.tile([D, BLK], F32, tag="ps")
                    mm(o_ps[:], W["wo2T"][:, n * D:(n + 1) * D], tp[:],
                       start=True, stop=False)
                    mm(o_ps[:], W["wovT"][:, n * D:(n + 1) * D], P[s1_][:],
                       start=False, stop=True)
                    u = wk_.tile([D, BLK], BF16, tag=f"u{n}", bufs=2)
                    nc.vector.scalar_tensor_tensor(
                        u[:], o_ps[:], W["ob2"][:, n:n + 1], P[n][:],
                        AL.add, AL.add)
                    us.append(u)
                st["u"] = us

            def s6_gate(st, b):
                r0 = (b % nblk) * BLK
                x2 = st["x2all"]
                g_ps = psp.tile([NB, BLK], F32, tag="ps")
                for n in range(NB):
                    mm(g_ps[:], W["gwT"][:, n * NB:(n + 1) * NB], x2[:, n, :],
                       start=(n == 0), stop=(n == 2))
                e_sb = wk_.tile([NB, BLK], BF16, tag="esb", bufs=1)
                nc.scalar.activation(e_sb[:], g_ps[:], AF.Exp,
                                     bias=W["gateb"][:NB, 0:1])
                zb_ps = psp.tile([NB, BLK], F32, tag="ps")
                mm(zb_ps[:], W["ones3"][:NB, :NB], e_sb[:])
                lnz = wk_.tile([NB, BLK], BF16, tag="lnz", bufs=1)
                nc.scalar.activation(lnz[:], zb_ps[:], AF.Ln)
                rz3 = wk_.tile([NB, BLK], BF16, tag="rz3", bufs=1)
                nc.scalar.activation(rz3[:], lnz[:], AF.Exp, scale=-1.0)
                en = wk_.tile([NB, BLK], BF16, tag="en", bufs=1)
                nc.vector.tensor_tensor(en[:], e_sb[:], rz3[:], AL.mult)
                d0 = wk_.tile([D, BLK], BF16, tag="d0", bufs=1)
                nc.gpsimd.tensor_tensor(d0[:], x2[:, 0, :], x2[:, 2, :],
                                        AL.subtract)
                d1 = wk_.tile([D, BLK], BF16, tag="d1", bufs=1)
                nc.gpsimd.tensor_tensor(d1[:], x2[:, 1, :], x2[:, 2, :],
                                        AL.subtract)
                eb0 = psp.tile([D, BLK], F32, tag="ps")
                mm(eb0[:], W["esel2"][:NB, 0:D], en[:])
                eb1 = psp.tile([D, BLK], F32, tag="ps")
                mm(eb1[:], W["esel2"][:NB, D:2 * D], en[:])
                m0 = wk_.tile([D, BLK], BF16, tag="m0", bufs=1)
                nc.vector.tensor_tensor(m0[:], d0[:], eb0[:], AL.mult)
                m1 = wk_.tile([D, BLK], BF16, tag="m1", bufs=1)
                nc.vector.tensor_tensor(m1[:], d1[:], eb1[:], AL.mult)
                sfu = wk_.tile([D, BLK], BF16, tag="sfu", bufs=1)
                nc.gpsimd.tensor_tensor(sfu[:], x2[:, 2, :], m0[:], AL.add)
                fused = wk_.tile([D, BLK], BF16, tag="fused", bufs=1)
                nc.vector.tensor_tensor(fused[:], sfu[:], m1[:], AL.add)
                nc.sync.dma_start(out[:, r0:r0 + BLK], fused[:])

            # software-pipelined emission, depth 7:
            #   t | S0 load
            #   t+1 | S1 proj + LN1
            #   t+2 | S2 attention (qk -> scores -> out-proj -> u)
            #   t+3 | S3 LN2
            #   t+4 | S4 FFN
            #   t+5 | S5 LN3
            #   t+6 | S6 gate + store
            # Per-tick emission order groups the ACT queue into
            # [sqrt-run][tanh+gelu][exp] (3 table loads per tick).
            total = nblk * repeat
            bstate = {}
            for t in range(total + 6):
                if t < total:
                    bstate[t] = s0_load(t)
                if 0 <= t - 1 < total:
                    s1_proj(bstate[t - 1])
                if 0 <= t - 2 < total:
                    s2a_qk(bstate[t - 2])
                if 0 <= t - 3 < total:
                    s3_ln2(bstate[t - 3])
                if 0 <= t - 6 < total:
                    s6_gate(bstate.pop(t - 6), t - 6)
                if 0 <= t - 5 < total:
                    s5_ln3(bstate[t - 5])
                if 0 <= t - 2 < total:
                    s2b_score(bstate[t - 2])
                if 0 <= t - 4 < total:
                    s4_ffn_mm(bstate[t - 4])
                if 0 <= t - 2 < total:
                    s2b_out(bstate[t - 2])
                if 0 <= t - 4 < total:
                    s4_ffn_post(bstate[t - 4])
    _fix_wait_overflow(nc)
    return nc


def kernel(**inputs):
    _patch_tile_drain()
    B = inputs["x_spatial"].shape[0]
    Bc = B // NCORES
    w = prep_weights(inputs)
    nc = build_program(Bc)
    xb = {k: np.ascontiguousarray(inputs[k]).astype(NPBF)
          for k in ("x_spatial", "x_gradient", "x_frequency")}
    in_maps = []
    for c in range(NCORES):
        m = dict(w)
        for k in ("x_spatial", "x_gradient", "x_frequency"):
            m[k] = np.ascontiguousarray(xb[k][c * Bc:(c + 1) * Bc])
        in_maps.append(m)
    res = run_bass_kernel_spmd(nc, in_maps, list(range(NCORES)))
    fm = np.concatenate([res.results[c]["out"] for c in range(NCORES)], axis=1)
    return np.ascontiguousarray(fm.T).astype(np.float32)


# revision 22
# speedup vs baseline: 1.7734x; 1.7734x over previous
"""Trainium2 Bass kernel for nn_CMAF (cross-modal attention fusion block).

Feature-major layout: every activation tile is [128 features x 1024
samples]; all matmuls are weight-stationary bf16 with batch as the
moving free dim.  Inputs are cast bf16 host-side and DMA-transposed in.

The elementwise work (not the PE) is the bottleneck for this model, so
ops are spread across all four compute engines per 1024-sample block:
  - ACT: Square(z+b), per-LN exp(-0.5*ln(var)), attention Tanh, Gelu,
    gate Exp + exp(-ln(Z)).  Everything lands in two ACT tables per
    block (natural_log_exp, then gelu+tanh) = 2 table loads/block.
    The s3/s5 LN groups share one wide Exp over a 3-slice tile.
  - DVE: PSUM-coupled fused ops (scalar_tensor_tensor), q PSUM->SBUF
    copy, gate normalize multiplies.
  - GPSIMD/Pool (otherwise idle): SBUF-only squares, LN applies, gate
    diffs.
  - PE: projections, q/k, dv, head-sum score matmul, out-proj, FFN, LN
    partition reductions (ones-matmul, on a dedicated 1-deep PSUM ring
    separate from the 3-deep data ring), gate broadcasts.

Algebraic folds (host-side, float64):
  - LayerNorm mean-subtraction folded into producing weights
    (C = I - 11^T/128); eps dropped (var ~ O(1) >> 1e-5).
  - 2-way attention softmax -> a0 = (1+tanh(q.dk/(2 sqrt(dh))))/2; the
    0.5 folded into Wo; the v1 path folded as Wov = C.Wo.Wv applied
    directly to P[s1], so v1 is never materialized.
  - LN rsqrt = exp(-0.5*ln(var)) on ACT: AF.Rsqrt is blocked in bass,
    custom-DVE ops fail this walrus, and hw reciprocal is ~6 cyc/elem.
  - gate softmax: fused = x2[2] + en0*(x2[0]-x2[2]) + en1*(x2[1]-x2[2]).

Output is written feature-major bf16 [128 x Bc]; the host transposes
back to [B, 128] f32 during the gather/unshard step.

Data parallel over 8 NeuronCores: 8192 samples each.
"""

import numpy as np
import ml_dtypes

import concourse.bass as bass
import concourse.mybir as mybir
from concourse.tile import TileContext
from concourse.vector_clock import ScopedClock
from concourse.bass_utils import run_bass_kernel_spmd

F32 = mybir.dt.float32
BF16 = mybir.dt.bfloat16
AL = mybir.AluOpType
AF = mybir.ActivationFunctionType
NPBF = ml_dtypes.bfloat16

D = 128
SP = 1280
FFN = 256
NB = 3
DH = 32
KV_IDX = ((1, 2), (0, 2), (0, 1))
NCORES = 8
BLK = 1024
MMN = 512
ISQ = float(1.0 / np.sqrt(DH))


def _patch_tile_drain():
    """walrus rejects >4 sem waits on one instruction; Tile's tail drain
    carries one wait per logical proc.  Re-emit them as standalone
    wait_ge instructions ahead of the drain."""
    TC = TileContext
    if getattr(TC, "_drain_patched", False):
        return

    def patched(self, tick_clock, wait_clock):
        nop_inst = self.nc.sync.nop()
        wait_clock.add_sem_waits(
            nop_inst.ins, ScopedClock({None: tick_clock.global_clock})
        )
        d = nop_inst.ins
        si = d.sync_info
        waits = list(si.on_wait) if si is not None else []
        if len(waits) > 4:
            si.on_wait = []
            d.sync_info = si
            name2sem = {s.name: s for s in self.sems.allocated().values()}
            for w in waits:
                sem = name2sem.get(w.ant_name)
                if sem is None:
                    raise RuntimeError(f"drain patch: unknown sem {w.ant_name}")
                self.nc.sync.wait_ge(sem, w.wait_value)
        self.nc.sync.drain()
        self.nc.all_engine_barrier()
        popped = self.nc._tile_sem_poison_stack.pop()
        assert popped is self._sem_poison
        self.nc.clear_and_free_semaphores(list(self.sems.allocated().values()))
        self.nc.all_engine_barrier()

    TC._drain_and_barrier = patched
    TC._drain_patched = True


def _fix_wait_overflow(nc):
    """walrus enforces per-opcode caps on sync-wait commands attached to
    one instruction.  Move the excess onto same-engine NOPs inserted
    immediately before the instruction."""
    LIMITS = {}
    DEFAULT_LIM = 1
    for fn in nc.m.functions:
        for bb in fn.blocks:
            insts = list(bb.instructions)
            out = []
            changed = False
            for inst in insts:
                si = getattr(inst, "sync_info", None)
                w = list(si.on_wait) if si is not None and si.on_wait else []
                lim = LIMITS.get(type(inst).__name__, DEFAULT_LIM)
                if len(w) > lim:
                    excess = w[lim:]
                    keep = w[:lim]
                    eng = nc.engines[inst.engine]
                    nops = []
                    for i in range(0, len(excess), 1):
                        chunk = excess[i:i + 1]
                        nop_bi = eng.nop()
                        nop_inst = nop_bi.ins
                        cb = nc.cur_bb.bb
                        cb.instructions = [x for x in cb.instructions
                                           if x.name != nop_inst.name]
                        import bass_rust
                        nop_inst.sync_info = bass_rust.SyncInfo(
                            on_wait=chunk, on_update=[])
                        nops.append(nop_inst)
                    si.on_wait = keep
                    inst.sync_info = si
                    out.extend(nops)
                    changed = True
                out.append(inst)
            if changed:
                bb.instructions = out


def prep_weights(inp):
    """Host-side prep of all weights into SBUF layouts. bf16 for matmul
    operands, fp32 for per-partition bias vectors."""
    f64 = np.float64
    C = np.eye(D, dtype=f64) - 1.0 / D

    def bf(a):
        return np.ascontiguousarray(a.astype(np.float32)).astype(NPBF)

    def f32(a):
        return np.ascontiguousarray(a, dtype=np.float32)

    w = {}
    wsp = C @ inp["proj_w_spatial"].astype(f64)            # [128,1280]
    w["wspT"] = bf(np.transpose(wsp.reshape(D, 10, D), (2, 1, 0)).reshape(D, 10 * D))
    wgf = np.stack([C @ inp["proj_w_gf"][i].astype(f64) for i in range(2)])
    w["wgfT"] = bf(np.transpose(wgf, (2, 0, 1)).reshape(D, 2 * D))
    w["bc"] = f32(C @ inp["proj_b"].astype(f64).T)         # [128,3]
    w["emb"] = f32(inp["mod_emb"].T)

    ipw = inp["in_proj_w"].astype(f64)                     # [3, 384, 128]
    wq, wk, wv = ipw[:, :D], ipw[:, D:2 * D], ipw[:, 2 * D:]
    w["wqT"] = bf(np.transpose(wq, (2, 0, 1)).reshape(D, NB * D))
    w["wkT"] = bf(np.transpose(wk, (2, 0, 1)).reshape(D, NB * D))
    w["wvT"] = bf(np.transpose(wv, (2, 0, 1)).reshape(D, NB * D))
    wo2 = np.stack([0.5 * (C @ inp["out_proj_w"][n].astype(f64))
                    for n in range(NB)])
    w["wo2T"] = bf(np.transpose(wo2, (2, 0, 1)).reshape(D, NB * D))
    wov = np.stack([C @ inp["out_proj_w"][n].astype(f64) @ wv[n]
                    for n in range(NB)])
    w["wovT"] = bf(np.transpose(wov, (2, 0, 1)).reshape(D, NB * D))
    ob2 = np.stack([
        C @ inp["out_proj_b"][n].astype(f64)
        - inp["mod_emb"][n].astype(f64).mean()
        for n in range(NB)])
    w["ob2"] = f32(ob2.T)

    w1 = inp["ffn_w1"].astype(f64)                         # [3, 256, 128]
    w["w1T"] = bf(np.transpose(w1, (2, 0, 1)).reshape(D, NB * FFN))
    w["b1"] = f32(inp["ffn_b1"].reshape(NB * 2, D).T)      # [128, 6]
    w2 = np.stack([C @ inp["ffn_w2"][n].astype(f64) for n in range(NB)])
    w2c = w2.reshape(NB, D, 2, D)                          # [n, j, c, p]
    w["w2T"] = bf(np.transpose(w2c, (3, 0, 2, 1)).reshape(D, NB * 2 * D))
    b2c = np.stack([C @ inp["ffn_b2"][n].astype(f64) for n in range(NB)])
    w["b2c"] = f32(b2c.T)

    gw = inp["gate_w"].astype(f64).reshape(NB, NB, D)      # [j, n, p]
    w["gwT"] = bf(np.transpose(gw, (2, 1, 0)).reshape(D, NB * NB))
    w["gateb"] = f32(inp["gate_b"].reshape(NB, 1))

    w["onesT"] = bf(np.full((D, D), 1.0 / D))
    hs = np.zeros((D, D), dtype=np.float32)
    for h in range(4):
        hs[h * DH:(h + 1) * DH, h * DH:(h + 1) * DH] = 1.0
    w["hsel"] = bf(hs)
    w["ones3"] = bf(np.ones((NB, NB)))
    esel2 = np.zeros((NB, 2 * D), dtype=np.float32)
    for n in range(2):
        esel2[n, n * D:(n + 1) * D] = 1.0
    w["esel2"] = bf(esel2)

    assert np.allclose(inp["proj_ln_g"], 1) and np.allclose(inp["proj_ln_b"], 0)
    assert np.allclose(inp["attn_ln_g"], 1) and np.allclose(inp["attn_ln_b"], 0)
    assert np.allclose(inp["ffn_ln_g"], 1) and np.allclose(inp["ffn_ln_b"], 0)
    assert np.allclose(inp["in_proj_b"], 0)
    return w


WEIGHT_SPECS = {
    "wspT": ((D, 10 * D), BF16), "wgfT": ((D, 2 * D), BF16),
    "bc": ((D, NB), F32), "emb": ((D, NB), F32),
    "wqT": ((D, NB * D), BF16), "wkT": ((D, NB * D), BF16),
    "wvT": ((D, NB * D), BF16), "wo2T": ((D, NB * D), BF16),
    "wovT": ((D, NB * D), BF16), "ob2": ((D, NB), F32),
    "w1T": ((D, NB * FFN), BF16), "b1": ((D, NB * 2), F32),
    "w2T": ((D, NB * 2 * D), BF16), "b2c": ((D, NB), F32),
    "gwT": ((D, NB * NB), BF16), "gateb": ((NB, 1), F32),
    "onesT": ((D, D), BF16), "hsel": ((D, D), BF16),
    "ones3": ((NB, NB), BF16), "esel2": ((NB, 2 * D), BF16),
}


def build_program(Bc, repeat=1):
    nc = bass.Bass()
    xsp = nc.dram_tensor("x_spatial", [Bc, SP], BF16, kind="ExternalInput")
    xg = nc.dram_tensor("x_gradient", [Bc, D], BF16, kind="ExternalInput")
    xf = nc.dram_tensor("x_frequency", [Bc, D], BF16, kind="ExternalInput")
    wd = {k: nc.dram_tensor(k, list(s[0]), s[1], kind="ExternalInput")
          for k, s in WEIGHT_SPECS.items()}
    out = nc.dram_tensor("out", [D, Bc], BF16, kind="ExternalOutput")

    nblk = Bc // BLK
    assert Bc % BLK == 0

    with TileContext(nc) as tc, nc.allow_low_precision(reason="bf16 kernel"):
        with (
            tc.tile_pool(name="wp", bufs=1) as wp,
            tc.tile_pool(name="xin", bufs=2) as xin,
            tc.tile_pool(name="work", bufs=2) as wk_,
            tc.tile_pool(name="ps", bufs=3, space="PSUM") as psp,
            tc.tile_pool(name="psm", bufs=1, space="PSUM") as psm,
        ):
            W = {}
            for k, s in WEIGHT_SPECS.items():
                W[k] = wp.tile(list(s[0]), s[1], tag=k, name=k)
                nc.gpsimd.dma_start(W[k][:], wd[k][:])

            def mm(out_ap, lhsT, rhs, start=True, stop=True):
                for h in range(BLK // MMN):
                    nc.tensor.matmul(out_ap[:, h * MMN:(h + 1) * MMN], lhsT,
                                     rhs[:, h * MMN:(h + 1) * MMN],
                                     start=start, stop=stop)

            def ln_group():
                """Per-stage LN scale pipeline: three Ln ops write slices of
                one [D, NB, BLK] tile; a single wide Exp computes all three
                1/sqrt(v) = exp(-0.5*ln(v)) broadcasts at once (saves two
                ACT issue overheads per stage; ln/exp stay in the
                natural_log_exp table -- hw reciprocal is ~6 cyc/elem)."""
                lnv = wk_.tile([D, NB, BLK], BF16, tag="rv", bufs=1)
                rb = wk_.tile([D, NB, BLK], BF16, tag="rb", bufs=1)

                def emit_ln(n, mq_ps):
                    nc.scalar.activation(lnv[:, n, :], mq_ps[:], AF.Ln)

                def emit_exp():
                    nc.scalar.activation(rb[:], lnv[:], AF.Exp, scale=-0.5)
                return rb, emit_ln, emit_exp

            def s0_load(b):
                r0 = (b % nblk) * BLK
                st = {}
                xspT = xin.tile([D, 10 * BLK], BF16, tag="xspT")
                nc.sync.dma_start(
                    xspT[:].rearrange("p (c n) -> p c n", c=10),
                    xsp[r0:r0 + BLK, :], transpose=True)
                st["xspT"] = xspT
                st["xgT"] = xin.tile([D, BLK], BF16, tag="xgT", name="xgT")
                nc.sync.dma_start(st["xgT"][:], xg[r0:r0 + BLK, :], transpose=True)
                st["xfT"] = xin.tile([D, BLK], BF16, tag="xfT", name="xfT")
                nc.sync.dma_start(st["xfT"][:], xf[r0:r0 + BLK, :], transpose=True)
                return st

            def s1_proj(st):
                P = [None] * NB
                for n in (1, 2, 0):
                    z = psp.tile([D, BLK], F32, tag="ps")
                    if n == 0:
                        for c in range(10):
                            mm(z[:], W["wspT"][:, c * D:(c + 1) * D],
                               st["xspT"][:, c * BLK:(c + 1) * BLK],
                               start=(c == 0), stop=(c == 9))
                    else:
                        key = "xgT" if n == 1 else "xfT"
                        mm(z[:], W["wgfT"][:, (n - 1) * D:n * D], st[key][:])
                    sqa = wk_.tile([D, BLK], BF16, tag="sqx", bufs=2)
                    nc.scalar.activation(sqa[:], z[:], AF.Square,
                                         bias=W["bc"][:, n:n + 1])
                    mq = psm.tile([D, BLK], F32, tag="mq")
                    mm(mq[:], W["onesT"][:], sqa[:])
                    lnv = wk_.tile([D, BLK], BF16, tag="rv1", bufs=1)
                    nc.scalar.activation(lnv[:], mq[:], AF.Ln)
                    rb = wk_.tile([D, BLK], BF16, tag="rb1", bufs=1)
                    nc.scalar.activation(rb[:], lnv[:], AF.Exp, scale=-0.5)
                    p_ = wk_.tile([D, BLK], BF16, tag=f"P{n}", bufs=2)
                    nc.vector.scalar_tensor_tensor(
                        p_[:], z[:], W["bc"][:, n:n + 1], rb[:],
                        AL.add, AL.mult)
                    nc.vector.tensor_scalar_add(p_[:], p_[:], W["emb"][:, n:n + 1])
                    P[n] = p_
                st["P"] = P
                dP = []
                for n in range(NB):
                    s0_, s1_ = KV_IDX[n]
                    dp = wk_.tile([D, BLK], BF16, tag=f"dP{n}", bufs=2)
                    nc.gpsimd.tensor_tensor(dp[:], P[s0_][:], P[s1_][:],
                                            AL.subtract)
                    dP.append(dp)
                st["dP"] = dP

            def s3_ln2(st):
                u = st["u"]
                rb2, emit_ln, emit_exp = ln_group()
                for n in range(NB):
                    sq2 = wk_.tile([D, BLK], BF16, tag="sqx", bufs=2)
                    nc.gpsimd.tensor_tensor(sq2[:], u[n][:], u[n][:], AL.mult)
                    mq2 = psm.tile([D, BLK], F32, tag="mq")
                    mm(mq2[:], W["onesT"][:], sq2[:])
                    emit_ln(n, mq2)
                emit_exp()
                x1 = []
                for n in range(NB):
                    x1n = wk_.tile([D, BLK], BF16, tag=f"x1{n}", bufs=2)
                    nc.gpsimd.tensor_tensor(x1n[:], u[n][:], rb2[:, n, :],
                                            AL.mult)
                    x1.append(x1n)
                st["x1"] = x1

            def s5_ln3(st):
                x2p, sq3 = st["x2p"], st["sq3"]
                x2all = wk_.tile([D, NB, BLK], BF16, tag="x2all", bufs=2)
                rb3, emit_ln, emit_exp = ln_group()
                for n in range(NB):
                    mq3 = psm.tile([D, BLK], F32, tag="mq")
                    mm(mq3[:], W["onesT"][:], sq3[n][:])
                    emit_ln(n, mq3)
                emit_exp()
                for n in range(NB):
                    nc.gpsimd.tensor_tensor(x2all[:, n, :], x2p[n][:],
                                            rb3[:, n, :], AL.mult)
                st["x2all"] = x2all

            def s2a_qk(st):
                P, dP = st["P"], st["dP"]
                t0s = []
                for n in range(NB):
                    q_ps = psp.tile([D, BLK], F32, tag="ps")
                    mm(q_ps[:], W["wqT"][:, n * D:(n + 1) * D], P[n][:])
                    dk_ps = psp.tile([D, BLK], F32, tag="ps")
                    mm(dk_ps[:], W["wkT"][:, n * D:(n + 1) * D], dP[n][:])
                    qsb = wk_.tile([D, BLK], BF16, tag="qsb", bufs=2)
                    nc.vector.tensor_copy(qsb[:], q_ps[:])
                    t0 = wk_.tile([D, BLK], BF16, tag="t0", bufs=3)
                    nc.vector.tensor_tensor(t0[:], qsb[:], dk_ps[:], AL.mult)
                    t0s.append(t0)
                st["t0"] = t0s

            def s4_ffn(st):
                x1 = st["x1"]
                x2p = []
                sq3 = []
                for n in range(NB):
                    h_sb = wk_.tile([D, 2, BLK], BF16, tag="hh", bufs=1)
                    for c in range(2):
                        h_ps = psp.tile([D, BLK], F32, tag="ps")
                        mm(h_ps[:],
                           W["w1T"][:, n * FFN + c * D: n * FFN + (c + 1) * D],
                           x1[n][:])
                        nc.scalar.activation(
                            h_sb[:, c, :], h_ps[:], AF.Gelu,
                            bias=W["b1"][:, 2 * n + c: 2 * n + c + 1])
                    f_ps = psp.tile([D, BLK], F32, tag="ps")
                    for c in range(2):
                        mm(f_ps[:], W["w2T"][:, (2 * n + c) * D:(2 * n + c + 1) * D],
                           h_sb[:, c, :], start=(c == 0), stop=(c == 1))
                    x2pn = wk_.tile([D, BLK], BF16, tag=f"x2p{n}", bufs=2)
                    nc.vector.scalar_tensor_tensor(
                        x2pn[:], f_ps[:], W["b2c"][:, n:n + 1], x1[n][:],
                        AL.add, AL.add)
                    x2p.append(x2pn)
                    sq3n = wk_.tile([D, BLK], BF16, tag=f"sq3{n}", bufs=2)
                    nc.gpsimd.tensor_tensor(sq3n[:], x2pn[:], x2pn[:], AL.mult)
                    sq3.append(sq3n)
                st["x2p"] = x2p
                st["sq3"] = sq3

            def s2b_score(st):
                t0s = st["t0"]
                ths = []
                for n in range(NB):
                    d_ps = psp.tile([D, BLK], F32, tag="ps")
                    mm(d_ps[:], W["hsel"][:], t0s[n][:])
                    th = wk_.tile([D, BLK], BF16, tag="th", bufs=3)
                    nc.scalar.activation(th[:], d_ps[:], AF.Tanh, scale=ISQ / 2)
                    ths.append(th)
                st["th"] = ths

            def s2b_out(st):
                P, dP, ths = st["P"], st["dP"], st["th"]
                us = []
                for n in range(NB):
                    s0_, s1_ = KV_IDX[n]
                    dv_ps = psp.tile([D, BLK], F32, tag="ps")
                    mm(dv_ps[:], W["wvT"][:, n * D:(n + 1) * D], dP[n][:])
                    tp = wk_.tile([D, BLK], BF16, tag="tp", bufs=2)
                    nc.vector.scalar_tensor_tensor(
                        tp[:], ths[n][:], 1.0, dv_ps[:], AL.add, AL.mult)
                    o_ps = psp.tile([D, BLK], F32, tag="ps")
                    mm(o_ps[:], W["wo2T"][:, n * D:(n + 1) * D], tp[:],
                       start=True, stop=False)
                    mm(o_ps[:], W["wovT"][:, n * D:(n + 1) * D], P[s1_][:],
                       start=False, stop=True)
                    u = wk_.tile([D, BLK], BF16, tag=f"u{n}", bufs=2)
                    nc.vector.scalar_tensor_tensor(
                        u[:], o_ps[:], W["ob2"][:, n:n + 1], P[n][:],
                        AL.add, AL.add)
                    us.append(u)
                st["u"] = us

            def s6_gate(st, b):
                r0 = (b % nblk) * BLK
                x2 = st["x2all"]
                g_ps = psp.tile([NB, BLK], F32, tag="ps")
                for n in range(NB):
                    mm(g_ps[:], W["gwT"][:, n * NB:(n + 1) * NB], x2[:, n, :],
                       start=(n == 0), stop=(n == 2))
                e_sb = wk_.tile([NB, BLK], BF16, tag="esb", bufs=1)
                nc.scalar.activation(e_sb[:], g_ps[:], AF.Exp,
                                     bias=W["gateb"][:NB, 0:1])
                zb_ps = psp.tile([NB, BLK], F32, tag="ps")
                mm(zb_ps[:], W["ones3"][:NB, :NB], e_sb[:])
                lnz = wk_.tile([NB, BLK], BF16, tag="lnz", bufs=1)
                nc.scalar.activation(lnz[:], zb_ps[:], AF.Ln)
                rz3 = wk_.tile([NB, BLK], BF16, tag="rz3", bufs=1)
                nc.scalar.activation(rz3[:], lnz[:], AF.Exp, scale=-1.0)
                en = wk_.tile([NB, BLK], BF16, tag="en", bufs=1)
                nc.vector.tensor_tensor(en[:], e_sb[:], rz3[:], AL.mult)
                d0 = wk_.tile([D, BLK], BF16, tag="d0", bufs=1)
                nc.gpsimd.tensor_tensor(d0[:], x2[:, 0, :], x2[:, 2, :],
                                        AL.subtract)
                d1 = wk_.tile([D, BLK], BF16, tag="d1", bufs=1)
                nc.gpsimd.tensor_tensor(d1[:], x2[:, 1, :], x2[:, 2, :],
                                        AL.subtract)
                eb0 = psp.tile([D, BLK], F32, tag="ps")
                mm(eb0[:], W["esel2"][:NB, 0:D], en[:])
                eb1 = psp.tile([D, BLK], F32, tag="ps")
                mm(eb1[:], W["esel2"][:NB, D:2 * D], en[:])
                m0 = wk_.tile([D, BLK], BF16, tag="m0", bufs=1)
                nc.vector.tensor_tensor(m0[:], d0[:], eb0[:], AL.mult)
                m1 = wk_.tile([D, BLK], BF16, tag="m1", bufs=1)
                nc.vector.tensor_tensor(m1[:], d1[:], eb1[:], AL.mult)
                sfu = wk_.tile([D, BLK], BF16, tag="sfu", bufs=1)
                nc.gpsimd.tensor_tensor(sfu[:], x2[:, 2, :], m0[:], AL.add)
                fused = wk_.tile([D, BLK], BF16, tag="fused", bufs=1)
                nc.vector.tensor_tensor(fused[:], sfu[:], m1[:], AL.add)
                nc.sync.dma_start(out[:, r0:r0 + BLK], fused[:])

            # software-pipelined emission, depth 7:
            #   t | S0 load
            #   t+1 | S1 proj + LN1
            #   t+2 | S2 attention (qk -> scores -> out-proj -> u)
            #   t+3 | S3 LN2
            #   t+4 | S4 FFN
            #   t+5 | S5 LN3
            #   t+6 | S6 gate + store
            # Per-tick emission order groups the ACT queue into
            # [sqrt-run][tanh+gelu][exp] (3 table loads per tick).
            total = nblk * repeat
            bstate = {}
            for t in range(total + 6):
                if t < total:
                    bstate[t] = s0_load(t)
                if 0 <= t - 1 < total:
                    s1_proj(bstate[t - 1])
                if 0 <= t - 2 < total:
                    s2a_qk(bstate[t - 2])
                if 0 <= t - 3 < total:
                    s3_ln2(bstate[t - 3])
                if 0 <= t - 6 < total:
                    s6_gate(bstate.pop(t - 6), t - 6)
                if 0 <= t - 5 < total:
                    s5_ln3(bstate[t - 5])
                if 0 <= t - 2 < total:
                    s2b_score(bstate[t - 2])
                if 0 <= t - 4 < total:
                    s4_ffn(bstate[t - 4])
                if 0 <= t - 2 < total:
                    s2b_out(bstate[t - 2])
    _fix_wait_overflow(nc)
    return nc


def kernel(**inputs):
    _patch_tile_drain()
    B = inputs["x_spatial"].shape[0]
    Bc = B // NCORES
    w = prep_weights(inputs)
    nc = build_program(Bc)
    xb = {k: np.ascontiguousarray(inputs[k]).astype(NPBF)
          for k in ("x_spatial", "x_gradient", "x_frequency")}
    in_maps = []
    for c in range(NCORES):
        m = dict(w)
        for k in ("x_spatial", "x_gradient", "x_frequency"):
            m[k] = np.ascontiguousarray(xb[k][c * Bc:(c + 1) * Bc])
        in_maps.append(m)
    res = run_bass_kernel_spmd(nc, in_maps, list(range(NCORES)))
    fm = np.concatenate([res.results[c]["out"] for c in range(NCORES)], axis=1)
    return np.ascontiguousarray(fm.T).astype(np.float32)
